# revision 1
# baseline (speedup 1.0000x reference)
"""DiGCNNet forward on 8 Trainium2 NeuronCores, data-parallel over batch.

Math (per batch b):
  adj = mean_t graph_sigs[b]                  # [30, 30]
  xw  = real[b] @ W                           # [30, 256]
  agg = adj^T @ xw + conv_bias                # [30, 256]
  h   = relu(agg)
  ns  = h @ pool_w + pool_b                   # [30]
  lg  = ns @ head_w^T + head_b                # [7]
  out = softmax(lg)

Device strategy per core (64 batches, processed in 16 groups of 4):
  - T-reduce as a PE matmul: ones^T(1/64) @ G with two batches stacked on the
    128 partitions (K=128), out PSUM [2, 900].
  - adj scatter: PSUM->SBUF copy (ACT) then SBUF->SBUF DMA [1,900] -> [30,30]
    diagonal blocks of a [121, 120] block-diagonal lhsT (row 120 = ones for
    the conv_bias contraction row).
  - xw: real loaded transposed via stride-1-partition DMA ([128(f), 4, 120(n)]),
    4 accumulating matmuls against pre-chunked W -> PSUM [120, 256].
  - agg: one block-diagonal matmul [121,120]^T @ [121,256] -> PSUM [120,256]
    (rhs row 120 = conv_bias).
  - relu on ACT, pool via one tensor_tensor_reduce (mult+add, init=pool_b).
  - head: constant block-diag head_w^T [120, 28] matmul -> logits [28, 1].
  - softmax tail on [28, 16] with 7-block partition sums done via tiny matmuls.
"""

from contextlib import ExitStack

import numpy as np

import concourse.bacc as bacc
import concourse.bass as bass
import concourse.tile as tile
from concourse import mybir
from concourse.bass_utils import run_bass_kernel_spmd

F32 = mybir.dt.float32
F32R = mybir.dt.float32r

B, T, N = 512, 64, 30
F_IN, D, C = 512, 256, 7
NCORES = 8
BL = B // NCORES        # 64 batches per core
GPB = 4                 # batches per group
NG = BL // GPB          # 16 groups
NN = N * N              # 900
NB = GPB * N            # 120 stacked node rows per group


def _build_nc():
    nc = bacc.Bacc(None, target_bir_lowering=False)

    gs = nc.dram_tensor("gs", (BL, T, N, N), F32, kind="ExternalInput")
    # real pre-transposed on host to [F_IN, BL*N] so chunk loads are
    # contiguous-innermost for the DMA engines.
    realt = nc.dram_tensor("realt", (F_IN, BL * N), F32, kind="ExternalInput")
    wt = nc.dram_tensor("wt", (128, 4, D), F32, kind="ExternalInput")
    cb = nc.dram_tensor("cb", (1, D), F32, kind="ExternalInput")
    pwb = nc.dram_tensor("pwb", (NB, D), F32, kind="ExternalInput")
    hwblk = nc.dram_tensor("hwblk", (NB, GPB * C), F32, kind="ExternalInput")
    hbb = nc.dram_tensor("hbb", (GPB * C, 1), F32, kind="ExternalInput")
    ones2 = nc.dram_tensor("ones2", (128, 2), F32, kind="ExternalInput")
    ones1 = nc.dram_tensor("ones1", (1, NB), F32, kind="ExternalInput")
    b7 = nc.dram_tensor("b7", (GPB * C, GPB), F32, kind="ExternalInput")
    b7t = nc.dram_tensor("b7t", (GPB, GPB * C), F32, kind="ExternalInput")
    out = nc.dram_tensor("out", (BL, C), F32, kind="ExternalOutput")

    with tile.TileContext(nc) as tc, ExitStack() as ctx:
        consts = ctx.enter_context(tc.tile_pool(name="consts", bufs=1))
        gt_pool = ctx.enter_context(tc.tile_pool(name="gt", bufs=8))
        adjs_pool = ctx.enter_context(tc.tile_pool(name="adjs", bufs=6))
        adjb_pool = ctx.enter_context(tc.tile_pool(name="adjb", bufs=16))
        xwb_pool = ctx.enter_context(tc.tile_pool(name="xwb", bufs=2))
        h_pool = ctx.enter_context(tc.tile_pool(name="h", bufs=2))
        scr_pool = ctx.enter_context(tc.tile_pool(name="scr", bufs=2))
        ns_pool = ctx.enter_context(tc.tile_pool(name="ns", bufs=2))
        tail_pool = ctx.enter_context(tc.tile_pool(name="tail", bufs=1))
        adjp_pool = ctx.enter_context(
            tc.tile_pool(name="adjp", bufs=2, space=bass.MemorySpace.PSUM)
        )
        xwp_pool = ctx.enter_context(
            tc.tile_pool(name="xwp", bufs=2, space=bass.MemorySpace.PSUM)
        )
        aggp_pool = ctx.enter_context(
            tc.tile_pool(name="aggp", bufs=1, space=bass.MemorySpace.PSUM)
        )
        smallp_pool = ctx.enter_context(
            tc.tile_pool(name="smallp", bufs=1, space=bass.MemorySpace.PSUM)
        )

        def load_const(dram, shape, dtype=F32):
            t = consts.tile(shape, dtype, tag=dram.name)
            src_ap = dram[:].bitcast(dtype) if dtype is not F32 else dram[:]
            nc.scalar.dma_start(t[:], src_ap)
            return t

        wt_sb = load_const(wt, [128, 4, D], F32R)
        cb_sb = load_const(cb, [1, D], F32R)
        pwb_sb = load_const(pwb, [NB, D])
        hw_sb = load_const(hwblk, [NB, GPB * C])
        hbb_sb = load_const(hbb, [GPB * C, 1])
        ones2_sb = load_const(ones2, [128, 2], F32R)
        ones1_sb = load_const(ones1, [1, NB], F32R)
        b7_sb = load_const(b7, [GPB * C, GPB])
        b7t_sb = load_const(b7t, [GPB, GPB * C])

        logits_all = consts.tile([GPB * C, NG], F32, tag="logits_all")

        # whole realt resident in SBUF: [128(f%128), 4(f//128), 1920(b*n)]
        rt_all = consts.tile([128, 4, BL * N], F32R, tag="rt_all")
        nc.sync.dma_start(
            rt_all[:], realt.rearrange("(c p) m -> p c m", p=128).bitcast(F32R)
        )

        # ---- phase A: T-reduce all groups into persistent block-diag tiles
        adjb_tiles = []
        for g in range(NG):
            adjb_t = adjb_pool.tile([NB, NB], F32R, tag="adjb")
            nc.vector.memset(adjb_t[:].bitcast(F32), 0.0)
            adjb_tiles.append(adjb_t)

        for g in range(NG):
            b0 = g * GPB
            adjs_tiles = []
            for p2 in range(2):
                bb = b0 + 2 * p2
                gtile = gt_pool.tile([128, NN], F32R, tag="gt")
                nc.sync.dma_start(
                    gtile[:],
                    gs[bb : bb + 2].rearrange("b t i j -> (b t) (i j)").bitcast(F32R),
                )
                adjp_t = adjp_pool.tile([2, NN], F32, tag="adjp")
                nc.tensor.matmul(
                    adjp_t[:, 0:512], ones2_sb[:], gtile[:, 0:512],
                    start=True, stop=True,
                )
                nc.tensor.matmul(
                    adjp_t[:, 512:NN], ones2_sb[:], gtile[:, 512:NN],
                    start=True, stop=True,
                )
                adjs_t = adjs_pool.tile([2, NN], F32, tag="adjs")
                if p2 == 0:
                    nc.scalar.copy(adjs_t[:], adjp_t[:])
                else:
                    nc.vector.tensor_copy(adjs_t[:], adjp_t[:])
                adjs_tiles.append(adjs_t)
            for k in range(GPB):
                nc.gpsimd.dma_start(
                    adjb_tiles[g][k * N : (k + 1) * N, k * N : (k + 1) * N],
                    adjs_tiles[k // 2][k % 2 : k % 2 + 1, :].bitcast(F32R),
                )

        # ---- phase B: xw -> agg -> relu -> pool -> head per group
        for g in range(NG):
            b0 = g * GPB
            xwp_t = xwp_pool.tile([NB, D], F32, tag="xwp")
            for c4 in range(4):
                nc.tensor.matmul(
                    xwp_t[:], rt_all[:, c4, b0 * N : (b0 + GPB) * N],
                    wt_sb[:, c4, :], start=(c4 == 0), stop=(c4 == 3),
                )
            xwb_t = xwb_pool.tile([NB, D], F32R, tag="xwb")
            nc.vector.tensor_copy(xwb_t[:], xwp_t[:])

            aggp_t = aggp_pool.tile([NB, D], F32, tag="aggp")
            nc.tensor.matmul(
                aggp_t[:], adjb_tiles[g][:], xwb_t[:], start=True, stop=False,
            )
            nc.tensor.matmul(
                aggp_t[:], ones1_sb[:], cb_sb[:], start=False, stop=True,
            )

            h_t = h_pool.tile([NB, D], F32, tag="h")
            nc.scalar.activation(h_t[:], aggp_t[:], mybir.ActivationFunctionType.Relu)
            scr_t = scr_pool.tile([NB, D], F32, tag="scr")
            ns_t = ns_pool.tile([NB, 1], F32, tag="ns")
            nc.vector.tensor_mul(scr_t[:], h_t[:], pwb_sb[:])
            nc.vector.reduce_sum(ns_t[:], scr_t[:], axis=mybir.AxisListType.X)

            lg_t = smallp_pool.tile([GPB * C, 1], F32, tag="small")
            nc.tensor.matmul(lg_t[:], hw_sb[:], ns_t[:], start=True, stop=True)
            nc.vector.tensor_add(logits_all[:, g : g + 1], lg_t[:], hbb_sb[:])

        # ---- softmax over the 7 classes (partition sub-blocks of 7)
        e_t = tail_pool.tile([GPB * C, NG], F32, tag="e")
        nc.scalar.activation(e_t[:], logits_all[:], mybir.ActivationFunctionType.Exp)
        sum_p = smallp_pool.tile([GPB, NG], F32, tag="small")
        nc.tensor.matmul(sum_p[:], b7_sb[:], e_t[:], start=True, stop=True)
        ssb_t = tail_pool.tile([GPB, NG], F32, tag="ssb")
        nc.vector.tensor_copy(ssb_t[:], sum_p[:])
        bcast_p = smallp_pool.tile([GPB * C, NG], F32, tag="small")
        nc.tensor.matmul(bcast_p[:], b7t_sb[:], ssb_t[:], start=True, stop=True)
        rs_t = tail_pool.tile([GPB * C, NG], F32, tag="rs")
        nc.vector.reciprocal(rs_t[:], bcast_p[:])
        res_t = tail_pool.tile([GPB * C, NG], F32, tag="res")
        nc.vector.tensor_mul(res_t[:], e_t[:], rs_t[:])
        nc.scalar.dma_start(out.rearrange("(g bi) c -> (bi c) g", bi=GPB), res_t[:])

    nc.compile()
    return nc


_NC_CACHE = None


def _get_nc():
    global _NC_CACHE
    if _NC_CACHE is None:
        _NC_CACHE = _build_nc()
    return _NC_CACHE


def _f32c(x):
    return np.ascontiguousarray(np.asarray(x, dtype=np.float32))


def _prepare_in_maps(real, graph_sigs, W, conv_bias, pool_w, pool_b, head_w, head_b):
    real = _f32c(real)
    graph_sigs = _f32c(graph_sigs)
    W = _f32c(W)

    wt = np.ascontiguousarray(
        _f32c(W).reshape(4, 128, D).transpose(1, 0, 2)
    )  # [128(f%128), 4(f//128), 256]
    cb = _f32c(conv_bias).reshape(1, D)
    pwb = np.ascontiguousarray(np.broadcast_to(_f32c(pool_w), (NB, D)))
    hw_t = _f32c(head_w).T  # [30, 7]
    hwblk = np.zeros((NB, GPB * C), dtype=np.float32)
    for k in range(GPB):
        hwblk[k * N : (k + 1) * N, k * C : (k + 1) * C] = hw_t
    # pool_b shifts every node score by a constant; fold it into the head
    # bias: logits[c] += pool_b * sum_j head_w[c, j]
    hb_eff = _f32c(head_b) + np.float32(np.asarray(pool_b)) * _f32c(head_w).sum(axis=1)
    hbb = np.tile(hb_eff, GPB).reshape(GPB * C, 1)
    ones2 = np.zeros((128, 2), dtype=np.float32)
    ones2[0:64, 0] = 1.0 / T
    ones2[64:128, 1] = 1.0 / T
    b7 = np.zeros((GPB * C, GPB), dtype=np.float32)
    for k in range(GPB):
        b7[k * C : (k + 1) * C, k] = 1.0
    b7t = np.ascontiguousarray(b7.T)
    ones1 = np.ones((1, NB), dtype=np.float32)

    consts = {
        "wt": wt, "cb": cb, "pwb": pwb, "hwblk": hwblk,
        "hbb": hbb, "ones2": ones2, "ones1": ones1, "b7": b7, "b7t": b7t,
    }
    in_maps = []
    for c in range(NCORES):
        s = slice(c * BL, (c + 1) * BL)
        in_maps.append(
            {
                "gs": np.ascontiguousarray(graph_sigs[s]),
                "realt": np.ascontiguousarray(
                    real[s].transpose(2, 0, 1).reshape(F_IN, BL * N)
                ),
                **consts,
            }
        )
    return in_maps


def kernel(real, imag, graph_sigs, W, conv_bias, pool_w, pool_b, head_w, head_b):
    del imag  # unused by the forward pass
    in_maps = _prepare_in_maps(
        real, graph_sigs, W, conv_bias, pool_w, pool_b, head_w, head_b
    )
    nc = _get_nc()
    res = run_bass_kernel_spmd(nc, in_maps, core_ids=list(range(NCORES)))
    return np.concatenate([res.results[c]["out"] for c in range(NCORES)], axis=0)



# revision 28
# speedup vs baseline: 1.6071x; 1.6071x over previous
"""DiGCNNet forward on 8 Trainium2 NeuronCores, data-parallel over batch.

Math (per batch b):
  adj = mean_t graph_sigs[b]                  # [30, 30]
  xw  = real[b] @ W                           # [30, 256]
  agg = adj^T @ xw + conv_bias                # [30, 256]
  h   = relu(agg)
  ns  = h @ pool_w + pool_b                   # [30]
  lg  = ns @ head_w^T + head_b                # [7]
  out = softmax(lg)

V2 design (64 batches/core, 16 gtiles of 4 batches, 4 quarters):
  - gs shipped bf16 from host (halves dominant DMA traffic; rel err ~4.5e-3).
  - T-reduce: PE matmul with per-gtile selector weights ones8 [128, 8]
    accumulating 4 gtiles into ONE PSUM tile [8, 1800] (4 banks), so the
    PSUM->SBUF copy is a single [8, 1800] op per quarter (not 16x [2,900]).
  - adjacency scatter: ONE reshape DMA per group (HWDGE, not gpsimd SWDGE):
    adjs[2 rows, 1800] -> Ablk[128part = (bo,bi,i), 32cols] fp32.
  - agg: per-batch matmuls on 32-aligned PE quadrants, K=31 row appends the
    conv_bias via a constant ones row in Ablk + bias rows in xwb (bias rows
    injected by the xwb PSUM->SBUF tensor_add with cbfull).
  - pool: W pre-scaled by |pool_w| and column-permuted (positives first) on
    host; relu+accum_out on ACT gives nsP/nsN per partition; ns = nsP-nsN
    realized inside the head matmul (rhs [128, 2], out [28, 2] per group).
  - head bias: folded into the exp() activation bias (per-partition AP).
  - softmax tail on [28, 16] once at the end.
"""

from contextlib import ExitStack

import numpy as np

import concourse.bacc as bacc
import concourse.bass as bass
import concourse.tile as tile
from concourse import mybir
from concourse.bass_utils import run_bass_kernel_spmd

F32 = mybir.dt.float32
F32R = mybir.dt.float32r
F16 = mybir.dt.float16
F16NP = np.float16

B, T, N = 512, 64, 30
F_IN, D, C = 512, 256, 7
NCORES = 8
BL = B // NCORES        # 64 batches per core
NN = N * N              # 900
NT = 16                 # 4-batch gtiles per core
NG = 16                 # groups of 4 batches
NP = 32                 # padded per-batch stride (partitions / realt cols)


def _build_nc(k):
    """k = number of (permuted-first) non-negative pool_w columns."""
    assert 1 <= k <= D - 1
    nc = bacc.Bacc(None, target_bir_lowering=False)

    # gs viewed as [32 gtiles of 2 batches, (b,t) partition, i*30+j free]
    gs = nc.dram_tensor("gs", (32, 128, NN), F16, kind="ExternalInput")
    rtp = nc.dram_tensor("rtp", (F_IN, BL * NP), F16, kind="ExternalInput")
    wt = nc.dram_tensor("wt", (128, 4, D), F16, kind="ExternalInput")
    ones16 = nc.dram_tensor("ones16", (128, 8, 16), F16, kind="ExternalInput")
    cbfull = nc.dram_tensor("cbfull", (128, D), F32, kind="ExternalInput")
    hwblk = nc.dram_tensor("hwblk", (128, 4 * C), F32, kind="ExternalInput")
    hbb = nc.dram_tensor("hbb", (4 * C, 1), F32, kind="ExternalInput")
    b7 = nc.dram_tensor("b7", (4 * C, 4), F32, kind="ExternalInput")
    b7t = nc.dram_tensor("b7t", (4, 4 * C), F32, kind="ExternalInput")
    out = nc.dram_tensor("out", (BL, C), F32, kind="ExternalOutput")

    with tile.TileContext(nc) as tc, ExitStack() as ctx:
        consts = ctx.enter_context(tc.tile_pool(name="consts", bufs=1))
        gt_pool = ctx.enter_context(tc.tile_pool(name="gt", bufs=12))

        xwb_pool = ctx.enter_context(tc.tile_pool(name="xwb", bufs=2))
        h_pool = ctx.enter_context(tc.tile_pool(name="h", bufs=2))
        ns_pool = ctx.enter_context(tc.tile_pool(name="ns", bufs=2))
        tail_pool = ctx.enter_context(tc.tile_pool(name="tail", bufs=1))
        adjp_pool = ctx.enter_context(
            tc.tile_pool(name="adjp", bufs=1, space=bass.MemorySpace.PSUM)
        )
        xwp_pool = ctx.enter_context(
            tc.tile_pool(name="xwp", bufs=2, space=bass.MemorySpace.PSUM)
        )
        aggp_pool = ctx.enter_context(
            tc.tile_pool(name="aggp", bufs=2, space=bass.MemorySpace.PSUM)
        )
        logp_pool = ctx.enter_context(
            tc.tile_pool(name="logp", bufs=1, space=bass.MemorySpace.PSUM)
        )

        def load_const(dram, shape, dtype):
            t = consts.tile(shape, dtype, tag=dram.name)
            src = dram[:].bitcast(dtype) if dtype is F32R else dram[:]
            nc.scalar.dma_start(t[:], src)
            return t

        # big realt load first on the scalar queue so it streams alongside
        # the first gtiles; needed by the first xw matmul (~7us in)
        rt_sb = consts.tile([128, 4, BL * NP], F16, tag="rt")
        nc.scalar.dma_start(rt_sb[:], rtp.rearrange("(c p) m -> p c m", p=128))

        wt_sb = load_const(wt, [128, 4, D], F16)
        ones16_sb = load_const(ones16, [128, 8, 16], F16)
        cb_sb = load_const(cbfull, [128, D], F32)
        hw_sb = load_const(hwblk, [128, 4 * C], F32)
        hbb_sb = load_const(hbb, [4 * C, 1], F32)
        b7_sb = load_const(b7, [4 * C, 4], F32)
        b7t_sb = load_const(b7t, [4, 4 * C], F32)

        # block-adjacency store: [128, 16 groups, 32]; all-ones memset gives
        # finite pad cols (30,31) for the M=32 agg matmuls
        ablk = consts.tile([128, NG, NP], F16, tag="ablk")
        nc.vector.memset(ablk[:], 1.0)

        # adjacency staging rows, padded to 32 elem-slots per i so the
        # per-group reshape DMA is a clean 2D->2D partition split; the pad
        # slots (1.0) become the K=31 conv_bias ones-rows in ablk
        adjs_a = consts.tile([16, NP * N], F16, tag="adjs_a")
        adjs_b = consts.tile([16, NP * N], F16, tag="adjs_b")
        nc.vector.memset(adjs_a[:, NN : NP * N], 1.0)
        nc.vector.memset(adjs_b[:, NN : NP * N], 1.0)

        logp_t = logp_pool.tile([4 * C, 2 * NG], F32, tag="logits")

        gtiles = []

        def emit_loads(q):
            for q2 in range(8):
                gt = gt_pool.tile([128, NN], F16, tag="gt")
                nc.sync.dma_start(gt[:], gs[8 * q + q2])
                gtiles.append(gt)

        def emit_tred(q, q2, adjp_t):
            gt = gtiles[8 * q + q2]
            for c0, c1 in ((0, 512), (512, NN)):
                nc.tensor.matmul(
                    adjp_t[:, c0:c1], ones16_sb[:, q2, :], gt[:, c0:c1],
                    start=(q2 == 0), stop=(q2 == 7),
                )

        def emit_adj_finish(q, adjp_t):
            adjs_t = adjs_a if q % 2 == 0 else adjs_b
            if q % 2 == 0:
                nc.scalar.copy(adjs_t[:, 0:NN], adjp_t[:])
            else:
                nc.vector.tensor_copy(adjs_t[:, 0:NN], adjp_t[:])
            for g2 in range(4):
                g = 4 * q + g2
                nc.scalar.dma_start(
                    ablk[:, g, 0:N], adjs_t[4 * g2 : 4 * g2 + 4, :]
                )

        def emit_group(g):
            xwp_t = xwp_pool.tile([128, D], F32, tag="xwp")
            for c4 in range(4):
                nc.tensor.matmul(
                    xwp_t[:], rt_sb[:, c4, 128 * g : 128 * (g + 1)], wt_sb[:, c4, :],
                    start=(c4 == 0), stop=(c4 == 3),
                )
            xwb_t = xwb_pool.tile([128, D], F16, tag="xwb")
            nc.vector.tensor_add(xwb_t[:], xwp_t[:], cb_sb[:])

            aggp_t = aggp_pool.tile([128, D], F32, tag="aggp")
            for b in range(4):
                p0 = NP * b
                nc.tensor.matmul(
                    aggp_t[p0 : p0 + NP, :],
                    ablk[p0 : p0 + 31, g, :],
                    xwb_t[p0 : p0 + 31, :],
                    start=True, stop=True, tile_position=(p0, p0),
                )
            h_t = h_pool.tile([128, D], F32, tag="h")
            ns_t = ns_pool.tile([128, 2], F32, tag="ns")
            nc.scalar.activation(
                h_t[:, 0:k], aggp_t[:, 0:k],
                mybir.ActivationFunctionType.Relu, accum_out=ns_t[:, 0:1],
            )
            nc.scalar.activation(
                h_t[:, k:D], aggp_t[:, k:D],
                mybir.ActivationFunctionType.Relu, accum_out=ns_t[:, 1:2],
            )
            nc.tensor.matmul(
                logp_t[:, 2 * g : 2 * g + 2], hw_sb[:], ns_t[:],
                start=True, stop=True,
            )

        # ---- pipelined emission ----
        emit_loads(0)
        emit_loads(1)
        for q in range(4):
            adjp_t = adjp_pool.tile([16, NN], F32, tag="adjp")
            for q2 in range(8):
                emit_tred(q, q2, adjp_t)
                if q >= 1 and q2 % 2 == 1:
                    emit_group(4 * (q - 1) + q2 // 2)
            emit_adj_finish(q, adjp_t)
            if q < 2:
                emit_loads(q + 2)
        for g2 in range(4):
            emit_group(12 + g2)

        # ---- softmax tail over the 7-class blocks ----
        lgs_t = tail_pool.tile([4 * C, 2 * NG], F32, tag="lgs")
        nc.vector.tensor_copy(lgs_t[:], logp_t[:])
        lgd_t = tail_pool.tile([4 * C, NG], F32, tag="lgd")
        nc.vector.tensor_sub(
            lgd_t[:],
            lgs_t[:].rearrange("p (g two) -> p two g", two=2)[:, 0, :],
            lgs_t[:].rearrange("p (g two) -> p two g", two=2)[:, 1, :],
        )
        e_t = tail_pool.tile([4 * C, NG], F32, tag="e")
        nc.scalar.activation(
            e_t[:], lgd_t[:], mybir.ActivationFunctionType.Exp, bias=hbb_sb[:],
        )
        # tail matmuls reuse sub-regions of the (already consumed) logits bank
        sum_p = logp_t[0:4, 0:NG]
        nc.tensor.matmul(sum_p, b7_sb[:], e_t[:], start=True, stop=True)
        ssb_t = tail_pool.tile([4, NG], F32, tag="ssb")
        nc.vector.tensor_copy(ssb_t[:], sum_p)
        bc_p = logp_t[:, NG : 2 * NG]
        nc.tensor.matmul(bc_p, b7t_sb[:], ssb_t[:], start=True, stop=True)
        rs_t = tail_pool.tile([4 * C, NG], F32, tag="rs")
        nc.vector.reciprocal(rs_t[:], bc_p)
        res_t = tail_pool.tile([4 * C, NG], F32, tag="res")
        nc.vector.tensor_mul(res_t[:], e_t[:], rs_t[:])
        nc.sync.dma_start(out.rearrange("(g bi) c -> (bi c) g", bi=4), res_t[:])

    nc.compile()
    return nc


_NC_CACHE = {}


def _get_nc(k):
    if k not in _NC_CACHE:
        _NC_CACHE[k] = _build_nc(k)
    return _NC_CACHE[k]


def _f32(x):
    return np.asarray(x, dtype=np.float32)


def _prepare(real, graph_sigs, W, conv_bias, pool_w, pool_b, head_w, head_b):
    real = _f32(real)
    graph_sigs = _f32(graph_sigs)
    W = _f32(W)
    conv_bias = _f32(conv_bias)
    pool_w = _f32(pool_w)
    head_w = _f32(head_w)
    head_b = _f32(head_b)

    # permute feature columns: non-negative pool_w first; fold |pool_w| into W
    nonneg = pool_w >= 0
    perm = np.argsort(~nonneg, kind="stable")
    k = int(nonneg.sum())
    apw = np.abs(pool_w)[perm]
    Wp = np.ascontiguousarray((W[:, perm] * apw[None, :]).astype(F16NP))
    cbp = (conv_bias[perm] * apw).astype(np.float32)

    wt = np.ascontiguousarray(Wp.reshape(4, 128, D).transpose(1, 0, 2))

    ones16 = np.zeros((2, 64, 8, 16), dtype=F16NP)
    for b in range(2):
        for q2 in range(8):
            ones16[b, :, q2, 2 * q2 + b] = F16NP(1.0 / T)
    ones16 = ones16.reshape(128, 8, 16)

    cbfull = np.zeros((128, D), dtype=np.float32)
    for b in range(4):
        cbfull[NP * b + N, :] = cbp

    hwblk = np.zeros((128, 4 * C), dtype=np.float32)
    for b in range(4):
        hwblk[NP * b : NP * b + N, C * b : C * (b + 1)] = head_w.T
    hb_eff = head_b + np.float32(np.asarray(pool_b)) * head_w.sum(axis=1)
    hbb = np.tile(hb_eff, 4).reshape(4 * C, 1).astype(np.float32)

    b7 = np.zeros((4 * C, 4), dtype=np.float32)
    for b in range(4):
        b7[C * b : C * (b + 1), b] = 1.0
    b7t = np.ascontiguousarray(b7.T)

    consts = {
        "wt": wt, "ones16": ones16, "cbfull": cbfull,
        "hwblk": hwblk, "hbb": hbb, "b7": b7, "b7t": b7t,
    }
    gs_bf = graph_sigs.astype(F16NP)
    in_maps = []
    for c in range(NCORES):
        s = slice(c * BL, (c + 1) * BL)
        rt = real[s].transpose(2, 0, 1)                      # [512, BL, 30]
        rtp = np.zeros((F_IN, BL, NP), dtype=F16NP)
        rtp[:, :, :N] = rt
        gsc = np.ascontiguousarray(gs_bf[s].reshape(32, 128, NN))
        in_maps.append(
            {
                "gs": gsc,
                "rtp": rtp.reshape(F_IN, BL * NP),
                **consts,
            }
        )
    return in_maps, k


def kernel(real, imag, graph_sigs, W, conv_bias, pool_w, pool_b, head_w, head_b):
    del imag  # unused by the forward pass
    in_maps, k = _prepare(
        real, graph_sigs, W, conv_bias, pool_w, pool_b, head_w, head_b
    )
    nc = _get_nc(k)
    res = run_bass_kernel_spmd(nc, in_maps, core_ids=list(range(NCORES)))
    return np.concatenate([res.results[c]["out"] for c in range(NCORES)], axis=0)


# revision 30
# speedup vs baseline: 1.8046x; 1.1229x over previous
"""DiGCNNet forward on 8 Trainium2 NeuronCores, data-parallel over batch.

Math (per batch b):
  adj = mean_t graph_sigs[b]                  # [30, 30]
  xw  = real[b] @ W                           # [30, 256]
  agg = adj^T @ xw + conv_bias                # [30, 256]
  h   = relu(agg)
  ns  = h @ pool_w + pool_b                   # [30]
  lg  = ns @ head_w^T + head_b                # [7]
  out = softmax(lg)

V2 design (64 batches/core, 16 gtiles of 4 batches, 4 quarters):
  - gs shipped bf16 from host (halves dominant DMA traffic; rel err ~4.5e-3).
  - T-reduce: PE matmul with per-gtile selector weights ones8 [128, 8]
    accumulating 4 gtiles into ONE PSUM tile [8, 1800] (4 banks), so the
    PSUM->SBUF copy is a single [8, 1800] op per quarter (not 16x [2,900]).
  - adjacency scatter: ONE reshape DMA per group (HWDGE, not gpsimd SWDGE):
    adjs[2 rows, 1800] -> Ablk[128part = (bo,bi,i), 32cols] fp32.
  - agg: per-batch matmuls on 32-aligned PE quadrants, K=31 row appends the
    conv_bias via a constant ones row in Ablk + bias rows in xwb (bias rows
    injected by the xwb PSUM->SBUF tensor_add with cbfull).
  - pool: W pre-scaled by |pool_w| and column-permuted (positives first) on
    host; relu+accum_out on ACT gives nsP/nsN per partition; ns = nsP-nsN
    realized inside the head matmul (rhs [128, 2], out [28, 2] per group).
  - head bias: folded into the exp() activation bias (per-partition AP).
  - softmax tail on [28, 16] once at the end.
"""

from contextlib import ExitStack

import numpy as np

import concourse.bacc as bacc
import concourse.bass as bass
import concourse.tile as tile
from concourse import mybir
from concourse.bass_utils import run_bass_kernel_spmd

F32 = mybir.dt.float32
F32R = mybir.dt.float32r
F16 = mybir.dt.float16
F16NP = np.float16

B, T, N = 512, 64, 30
F_IN, D, C = 512, 256, 7
NCORES = 8
BL = B // NCORES        # 64 batches per core
NN = N * N              # 900
NT = 16                 # 4-batch gtiles per core
NG = 16                 # groups of 4 batches
NP = 32                 # padded per-batch stride (partitions / realt cols)


def _build_nc(k):
    """k = number of (permuted-first) non-negative pool_w columns."""
    assert 1 <= k <= D - 1
    nc = bacc.Bacc(None, target_bir_lowering=False)

    # gs pre-tiled: [16 loads, 128 = (b&1,t) partition, (gtile-pair, i*30+j)]
    gs = nc.dram_tensor("gs", (16, 128, 2 * NN), F16, kind="ExternalInput")
    rtp = nc.dram_tensor("rtp", (F_IN, BL * NP), F16, kind="ExternalInput")
    wo = nc.dram_tensor("wo", (128, 4 * D + 128), F16, kind="ExternalInput")
    cbhw = nc.dram_tensor("cbhw", (128, D + 4 * C), F32, kind="ExternalInput")
    h5 = nc.dram_tensor("h5", (4 * C, 5), F32, kind="ExternalInput")
    b7t = nc.dram_tensor("b7t", (4, 4 * C), F32, kind="ExternalInput")
    out = nc.dram_tensor("out", (BL, C), F32, kind="ExternalOutput")

    with tile.TileContext(nc) as tc, ExitStack() as ctx:
        consts = ctx.enter_context(tc.tile_pool(name="consts", bufs=1))
        gt_pool = ctx.enter_context(tc.tile_pool(name="gt", bufs=12))

        xwb_pool = ctx.enter_context(tc.tile_pool(name="xwb", bufs=2))
        h_pool = ctx.enter_context(tc.tile_pool(name="h", bufs=2))
        ns_pool = ctx.enter_context(tc.tile_pool(name="ns", bufs=2))
        tail_pool = ctx.enter_context(tc.tile_pool(name="tail", bufs=1))
        adjp_pool = ctx.enter_context(
            tc.tile_pool(name="adjp", bufs=1, space=bass.MemorySpace.PSUM)
        )
        xwp_pool = ctx.enter_context(
            tc.tile_pool(name="xwp", bufs=2, space=bass.MemorySpace.PSUM)
        )
        aggp_pool = ctx.enter_context(
            tc.tile_pool(name="aggp", bufs=2, space=bass.MemorySpace.PSUM)
        )
        logp_pool = ctx.enter_context(
            tc.tile_pool(name="logp", bufs=1, space=bass.MemorySpace.PSUM)
        )

        def load_const(dram, shape, dtype):
            t = consts.tile(shape, dtype, tag=dram.name)
            src = dram[:].bitcast(dtype) if dtype is F32R else dram[:]
            nc.scalar.dma_start(t[:], src)
            return t

        # small consts first (the first T-reduce matmul needs ones16);
        # the big rt load follows on the same queue
        wo_sb = load_const(wo, [128, 4 * D + 128], F16)
        cbhw_sb = load_const(cbhw, [128, D + 4 * C], F32)
        h5_sb = load_const(h5, [4 * C, 5], F32)
        b7t_sb = load_const(b7t, [4, 4 * C], F32)
        wt_sb = wo_sb[:, 0 : 4 * D].rearrange("p (c d) -> p c d", c=4)
        ones16_sb = wo_sb[:, 4 * D : 4 * D + 128].rearrange("p (q m) -> p q m", q=8)
        cb_sb = cbhw_sb[:, 0:D]
        hw_sb = cbhw_sb[:, D : D + 4 * C]
        hbb_sb = h5_sb[:, 0:1]
        b7_sb = h5_sb[:, 1:5]

        rt_sb = consts.tile([128, 4, BL * NP], F16, tag="rt")
        nc.scalar.dma_start(rt_sb[:], rtp.rearrange("(c p) m -> p c m", p=128))

        # block-adjacency store: [128, 16 groups, 32]; all-ones memset gives
        # finite pad cols (30,31) for the M=32 agg matmuls
        ablk = consts.tile([128, NG, NP], F16, tag="ablk")
        nc.vector.memset(ablk[:], 1.0)

        # adjacency staging rows, padded to 32 elem-slots per i so the
        # per-group reshape DMA is a clean 2D->2D partition split; the pad
        # slots (1.0) become the K=31 conv_bias ones-rows in ablk
        adjs_a = consts.tile([16, NP * N], F16, tag="adjs_a")
        adjs_b = consts.tile([16, NP * N], F16, tag="adjs_b")
        nc.vector.memset(adjs_a[:, NN : NP * N], 1.0)
        nc.vector.memset(adjs_b[:, NN : NP * N], 1.0)

        logp_t = logp_pool.tile([4 * C, 2 * NG], F32, tag="logits")

        gtiles = []

        def emit_loads(q):
            for u in range(4):
                gt = gt_pool.tile([128, 2 * NN], F16, tag="gt")
                nc.sync.dma_start(gt[:], gs[4 * q + u])
                gtiles.append(gt)

        def emit_tred(q, q2, adjp_t):
            gt = gtiles[4 * q + q2 // 2]
            o = (q2 % 2) * NN
            for c0, c1 in ((0, 512), (512, NN)):
                nc.tensor.matmul(
                    adjp_t[:, c0:c1], ones16_sb[:, q2, :], gt[:, o + c0 : o + c1],
                    start=(q2 == 0), stop=(q2 == 7),
                )

        def emit_adj_finish(q, adjp_t):
            adjs_t = adjs_a if q % 2 == 0 else adjs_b
            if q % 2 == 0:
                nc.scalar.copy(adjs_t[:, 0:NN], adjp_t[:])
            else:
                nc.vector.tensor_copy(adjs_t[:, 0:NN], adjp_t[:])
            for g2 in range(4):
                g = 4 * q + g2
                eng = nc.scalar if g2 % 2 == 0 else nc.sync
                eng.dma_start(ablk[:, g, 0:N], adjs_t[4 * g2 : 4 * g2 + 4, :])

        def emit_group(g):
            xwp_t = xwp_pool.tile([128, D], F32, tag="xwp")
            for c4 in range(4):
                nc.tensor.matmul(
                    xwp_t[:], rt_sb[:, c4, 128 * g : 128 * (g + 1)], wt_sb[:, c4, :],
                    start=(c4 == 0), stop=(c4 == 3),
                )
            xwb_t = xwb_pool.tile([128, D], F16, tag="xwb")
            nc.vector.tensor_add(xwb_t[:], xwp_t[:], cb_sb[:])

            aggp_t = aggp_pool.tile([128, D], F32, tag="aggp")
            for b in range(4):
                p0 = NP * b
                nc.tensor.matmul(
                    aggp_t[p0 : p0 + NP, :],
                    ablk[p0 : p0 + 31, g, :],
                    xwb_t[p0 : p0 + 31, :],
                    start=True, stop=True, tile_position=(p0, p0),
                )
            h_t = h_pool.tile([128, D], F32, tag="h")
            ns_t = ns_pool.tile([128, 2], F32, tag="ns")
            nc.scalar.activation(
                h_t[:, 0:k], aggp_t[:, 0:k],
                mybir.ActivationFunctionType.Relu, accum_out=ns_t[:, 0:1],
            )
            nc.vector.tensor_scalar(
                h_t[:, k:D], aggp_t[:, k:D], 0.0, 0.0,
                mybir.AluOpType.max, mybir.AluOpType.add,
                accum_out=ns_t[:, 1:2],
            )
            nc.tensor.matmul(
                logp_t[:, 2 * g : 2 * g + 2], hw_sb[:], ns_t[:],
                start=True, stop=True,
            )

        # ---- pipelined emission ----
        emit_loads(0)
        emit_loads(1)
        for q in range(4):
            adjp_t = adjp_pool.tile([16, NN], F32, tag="adjp")
            for q2 in range(8):
                emit_tred(q, q2, adjp_t)
                if q >= 1 and q2 % 2 == 1:
                    emit_group(4 * (q - 1) + q2 // 2)
            emit_adj_finish(q, adjp_t)
            if q < 2:
                emit_loads(q + 2)
        for g2 in range(4):
            emit_group(12 + g2)

        # ---- softmax tail over the 7-class blocks ----
        lgs_t = tail_pool.tile([4 * C, 2 * NG], F32, tag="lgs")
        nc.vector.tensor_copy(lgs_t[:], logp_t[:])
        lgd_t = tail_pool.tile([4 * C, NG], F32, tag="lgd")
        nc.vector.tensor_sub(
            lgd_t[:],
            lgs_t[:].rearrange("p (g two) -> p two g", two=2)[:, 0, :],
            lgs_t[:].rearrange("p (g two) -> p two g", two=2)[:, 1, :],
        )
        e_t = tail_pool.tile([4 * C, NG], F32, tag="e")
        nc.scalar.activation(
            e_t[:], lgd_t[:], mybir.ActivationFunctionType.Exp, bias=hbb_sb[:],
        )
        # tail matmuls reuse sub-regions of the (already consumed) logits bank
        sum_p = logp_t[0:4, 0:NG]
        nc.tensor.matmul(sum_p, b7_sb[:], e_t[:], start=True, stop=True)
        ssb_t = tail_pool.tile([4, NG], F32, tag="ssb")
        nc.vector.tensor_copy(ssb_t[:], sum_p)
        bc_p = logp_t[:, NG : 2 * NG]
        nc.tensor.matmul(bc_p, b7t_sb[:], ssb_t[:], start=True, stop=True)
        rs_t = tail_pool.tile([4 * C, NG], F32, tag="rs")
        nc.vector.reciprocal(rs_t[:], bc_p)
        res_t = tail_pool.tile([4 * C, NG], F32, tag="res")
        nc.vector.tensor_mul(res_t[:], e_t[:], rs_t[:])
        nc.sync.dma_start(out.rearrange("(g bi) c -> (bi c) g", bi=4), res_t[:])

    nc.compile()
    return nc


_NC_CACHE = {}


def _get_nc(k):
    if k not in _NC_CACHE:
        _NC_CACHE[k] = _build_nc(k)
    return _NC_CACHE[k]


def _f32(x):
    return np.asarray(x, dtype=np.float32)


def _prepare(real, graph_sigs, W, conv_bias, pool_w, pool_b, head_w, head_b):
    real = _f32(real)
    graph_sigs = _f32(graph_sigs)
    W = _f32(W)
    conv_bias = _f32(conv_bias)
    pool_w = _f32(pool_w)
    head_w = _f32(head_w)
    head_b = _f32(head_b)

    # permute feature columns: non-negative pool_w first; fold |pool_w| into W
    nonneg = pool_w >= 0
    perm = np.argsort(~nonneg, kind="stable")
    k = int(nonneg.sum())
    apw = np.abs(pool_w)[perm]
    Wp = np.ascontiguousarray((W[:, perm] * apw[None, :]).astype(F16NP))
    cbp = (conv_bias[perm] * apw).astype(np.float32)

    wt = np.ascontiguousarray(Wp.reshape(4, 128, D).transpose(1, 0, 2))

    ones16 = np.zeros((2, 64, 8, 16), dtype=F16NP)
    for b in range(2):
        for q2 in range(8):
            ones16[b, :, q2, 2 * q2 + b] = F16NP(1.0 / T)
    ones16 = ones16.reshape(128, 8, 16)

    cbfull = np.zeros((128, D), dtype=np.float32)
    for b in range(4):
        cbfull[NP * b + N, :] = cbp

    hwblk = np.zeros((128, 4 * C), dtype=np.float32)
    for b in range(4):
        hwblk[NP * b : NP * b + N, C * b : C * (b + 1)] = head_w.T
    hb_eff = head_b + np.float32(np.asarray(pool_b)) * head_w.sum(axis=1)
    hbb = np.tile(hb_eff, 4).reshape(4 * C, 1).astype(np.float32)

    b7 = np.zeros((4 * C, 4), dtype=np.float32)
    for b in range(4):
        b7[C * b : C * (b + 1), b] = 1.0
    b7t = np.ascontiguousarray(b7.T)

    wo = np.concatenate([wt.reshape(128, 4 * D), ones16.reshape(128, 128)], axis=1)
    cbhw = np.concatenate([cbfull, hwblk], axis=1)
    h5 = np.concatenate([hbb, b7], axis=1)
    consts = {"wo": wo, "cbhw": cbhw, "h5": h5, "b7t": b7t}
    gs_bf = graph_sigs.astype(F16NP)
    in_maps = []
    for c in range(NCORES):
        s = slice(c * BL, (c + 1) * BL)
        rt = real[s].transpose(2, 0, 1)                      # [512, BL, 30]
        rtp = np.zeros((F_IN, BL, NP), dtype=F16NP)
        rtp[:, :, :N] = rt
        gsc = np.ascontiguousarray(
            gs_bf[s]
            .reshape(16, 2, 2, T, NN)
            .transpose(0, 2, 3, 1, 4)
            .reshape(16, 128, 2 * NN)
        )
        in_maps.append(
            {
                "gs": gsc,
                "rtp": rtp.reshape(F_IN, BL * NP),
                **consts,
            }
        )
    return in_maps, k


def kernel(real, imag, graph_sigs, W, conv_bias, pool_w, pool_b, head_w, head_b):
    del imag  # unused by the forward pass
    in_maps, k = _prepare(
        real, graph_sigs, W, conv_bias, pool_w, pool_b, head_w, head_b
    )
    nc = _get_nc(k)
    res = run_bass_kernel_spmd(nc, in_maps, core_ids=list(range(NCORES)))
    return np.concatenate([res.results[c]["out"] for c in range(NCORES)], axis=0)


# revision 31
# speedup vs baseline: 1.9321x; 1.0707x over previous
"""DiGCNNet forward on 8 Trainium2 NeuronCores, data-parallel over batch.

Math (per batch b):
  adj = mean_t graph_sigs[b]                  # [30, 30]
  xw  = real[b] @ W                           # [30, 256]
  agg = adj^T @ xw + conv_bias                # [30, 256]
  h   = relu(agg)
  ns  = h @ pool_w + pool_b                   # [30]
  lg  = ns @ head_w^T + head_b                # [7]
  out = softmax(lg)

V2 design (64 batches/core, 16 gtiles of 4 batches, 4 quarters):
  - gs shipped bf16 from host (halves dominant DMA traffic; rel err ~4.5e-3).
  - T-reduce: PE matmul with per-gtile selector weights ones8 [128, 8]
    accumulating 4 gtiles into ONE PSUM tile [8, 1800] (4 banks), so the
    PSUM->SBUF copy is a single [8, 1800] op per quarter (not 16x [2,900]).
  - adjacency scatter: ONE reshape DMA per group (HWDGE, not gpsimd SWDGE):
    adjs[2 rows, 1800] -> Ablk[128part = (bo,bi,i), 32cols] fp32.
  - agg: per-batch matmuls on 32-aligned PE quadrants, K=31 row appends the
    conv_bias via a constant ones row in Ablk + bias rows in xwb (bias rows
    injected by the xwb PSUM->SBUF tensor_add with cbfull).
  - pool: W pre-scaled by |pool_w| and column-permuted (positives first) on
    host; relu+accum_out on ACT gives nsP/nsN per partition; ns = nsP-nsN
    realized inside the head matmul (rhs [128, 2], out [28, 2] per group).
  - head bias: folded into the exp() activation bias (per-partition AP).
  - softmax tail on [28, 16] once at the end.
"""

from contextlib import ExitStack

import numpy as np

import concourse.bacc as bacc
import concourse.bass as bass
import concourse.tile as tile
from concourse import mybir
from concourse.bass_utils import run_bass_kernel_spmd

F32 = mybir.dt.float32
F32R = mybir.dt.float32r
F16 = mybir.dt.float16
F8 = mybir.dt.float8e4
F16NP = np.float16
import ml_dtypes
F8NP = ml_dtypes.float8_e4m3

B, T, N = 512, 64, 30
F_IN, D, C = 512, 256, 7
NCORES = 8
BL = B // NCORES        # 64 batches per core
NN = N * N              # 900
NT = 16                 # 4-batch gtiles per core
NG = 16                 # groups of 4 batches
NP = 32                 # padded per-batch stride (partitions / realt cols)


def _build_nc(k):
    """k = number of (permuted-first) non-negative pool_w columns."""
    assert 1 <= k <= D - 1
    nc = bacc.Bacc(None, target_bir_lowering=False)

    # gs pre-tiled fp8: [8 loads, 128=(b&1,t) part, (h, u-pair, i*30+j)]
    gs = nc.dram_tensor("gs", (8, 128, 2, 2, NN), F8, kind="ExternalInput")
    rtp = nc.dram_tensor("rtp", (F_IN, BL * NP), F16, kind="ExternalInput")
    wo = nc.dram_tensor("wo", (128, 4 * D + 64), F16, kind="ExternalInput")
    cbhw = nc.dram_tensor("cbhw", (128, D + 4 * C), F32, kind="ExternalInput")
    h5 = nc.dram_tensor("h5", (4 * C, 5), F32, kind="ExternalInput")
    b7t = nc.dram_tensor("b7t", (4, 4 * C), F32, kind="ExternalInput")
    out = nc.dram_tensor("out", (BL, C), F32, kind="ExternalOutput")

    with tile.TileContext(nc) as tc, ExitStack() as ctx:
        consts = ctx.enter_context(tc.tile_pool(name="consts", bufs=1))
        gt_pool = ctx.enter_context(tc.tile_pool(name="gt", bufs=12))

        xwb_pool = ctx.enter_context(tc.tile_pool(name="xwb", bufs=2))
        h_pool = ctx.enter_context(tc.tile_pool(name="h", bufs=2))
        ns_pool = ctx.enter_context(tc.tile_pool(name="ns", bufs=2))
        tail_pool = ctx.enter_context(tc.tile_pool(name="tail", bufs=1))
        adjp_pool = ctx.enter_context(
            tc.tile_pool(name="adjp", bufs=1, space=bass.MemorySpace.PSUM)
        )
        xwp_pool = ctx.enter_context(
            tc.tile_pool(name="xwp", bufs=2, space=bass.MemorySpace.PSUM)
        )
        aggp_pool = ctx.enter_context(
            tc.tile_pool(name="aggp", bufs=2, space=bass.MemorySpace.PSUM)
        )
        logp_pool = ctx.enter_context(
            tc.tile_pool(name="logp", bufs=1, space=bass.MemorySpace.PSUM)
        )

        def load_const(dram, shape, dtype):
            t = consts.tile(shape, dtype, tag=dram.name)
            src = dram[:].bitcast(dtype) if dtype is F32R else dram[:]
            nc.scalar.dma_start(t[:], src)
            return t

        # small consts first (the first T-reduce matmul needs ones16);
        # the big rt load follows on the same queue
        wo_sb = load_const(wo, [128, 4 * D + 64], F16)
        cbhw_sb = load_const(cbhw, [128, D + 4 * C], F32)
        h5_sb = load_const(h5, [4 * C, 5], F32)
        b7t_sb = load_const(b7t, [4, 4 * C], F32)
        wt_sb = wo_sb[:, 0 : 4 * D].rearrange("p (c d) -> p c d", c=4)
        ones_dr = wo_sb[:, 4 * D : 4 * D + 64].bitcast(F8).rearrange(
            "p (u h m) -> p u h m", u=4, h=2
        )
        cb_sb = cbhw_sb[:, 0:D]
        hw_sb = cbhw_sb[:, D : D + 4 * C]
        hbb_sb = h5_sb[:, 0:1]
        b7_sb = h5_sb[:, 1:5]

        rt_sb = consts.tile([128, 4, BL * NP], F16, tag="rt")
        nc.scalar.dma_start(rt_sb[:], rtp.rearrange("(c p) m -> p c m", p=128))

        # block-adjacency store: [128, 16 groups, 32]; all-ones memset gives
        # finite pad cols (30,31) for the M=32 agg matmuls
        ablk = consts.tile([128, NG, NP], F16, tag="ablk")
        nc.vector.memset(ablk[:], 1.0)

        # adjacency staging rows, padded to 32 elem-slots per i so the
        # per-group reshape DMA is a clean 2D->2D partition split; the pad
        # slots (1.0) become the K=31 conv_bias ones-rows in ablk
        adjs_a = consts.tile([16, NP * N], F16, tag="adjs_a")
        adjs_b = consts.tile([16, NP * N], F16, tag="adjs_b")
        nc.vector.memset(adjs_a[:, NN : NP * N], 1.0)
        nc.vector.memset(adjs_b[:, NN : NP * N], 1.0)

        logp_t = logp_pool.tile([4 * C, 2 * NG], F32, tag="logits")

        gtiles = []

        def emit_loads(q):
            for v in range(2):
                gt = gt_pool.tile([128, 2, 2, NN], F8, tag="gt")
                nc.sync.dma_start(gt[:], gs[2 * q + v])
                gtiles.append(gt)

        def emit_tred(q, u2, adjp_t):
            gt = gtiles[2 * q + u2 // 2]
            uu = u2 % 2
            for c0, c1 in ((0, 512), (512, NN)):
                nc.tensor.matmul(
                    adjp_t[:, c0:c1], ones_dr[:, u2, :, :], gt[:, :, uu, c0:c1],
                    start=(u2 == 0), stop=(u2 == 3),
                    perf_mode=mybir.MatmulPerfMode.DoubleRow,
                )

        def emit_adj_finish(q, adjp_t):
            adjs_t = adjs_a if q % 2 == 0 else adjs_b
            if q % 2 == 0:
                nc.scalar.copy(adjs_t[:, 0:NN], adjp_t[:])
            else:
                nc.vector.tensor_copy(adjs_t[:, 0:NN], adjp_t[:])
            for g2 in range(4):
                g = 4 * q + g2
                eng = nc.scalar if g2 % 2 == 0 else nc.sync
                eng.dma_start(ablk[:, g, 0:N], adjs_t[4 * g2 : 4 * g2 + 4, :])

        def emit_group(g):
            xwp_t = xwp_pool.tile([128, D], F32, tag="xwp")
            for c4 in range(4):
                nc.tensor.matmul(
                    xwp_t[:], rt_sb[:, c4, 128 * g : 128 * (g + 1)], wt_sb[:, c4, :],
                    start=(c4 == 0), stop=(c4 == 3),
                )
            xwb_t = xwb_pool.tile([128, D], F16, tag="xwb")
            nc.vector.tensor_add(xwb_t[:], xwp_t[:], cb_sb[:])

            aggp_t = aggp_pool.tile([128, D], F32, tag="aggp")
            for b in range(4):
                p0 = NP * b
                nc.tensor.matmul(
                    aggp_t[p0 : p0 + NP, :],
                    ablk[p0 : p0 + 31, g, :],
                    xwb_t[p0 : p0 + 31, :],
                    start=True, stop=True, tile_position=(p0, p0),
                )
            h_t = h_pool.tile([128, D], F32, tag="h")
            ns_t = ns_pool.tile([128, 2], F32, tag="ns")
            nc.scalar.activation(
                h_t[:, 0:k], aggp_t[:, 0:k],
                mybir.ActivationFunctionType.Relu, accum_out=ns_t[:, 0:1],
            )
            nc.vector.tensor_scalar(
                h_t[:, k:D], aggp_t[:, k:D], 0.0, 0.0,
                mybir.AluOpType.max, mybir.AluOpType.add,
                accum_out=ns_t[:, 1:2],
            )
            nc.tensor.matmul(
                logp_t[:, 2 * g : 2 * g + 2], hw_sb[:], ns_t[:],
                start=True, stop=True,
            )

        # ---- pipelined emission ----
        emit_loads(0)
        emit_loads(1)
        for q in range(4):
            adjp_t = adjp_pool.tile([16, NN], F32, tag="adjp")
            for u2 in range(4):
                emit_tred(q, u2, adjp_t)
                if q >= 1:
                    emit_group(4 * (q - 1) + u2)
            emit_adj_finish(q, adjp_t)
            if q < 2:
                emit_loads(q + 2)
        for g2 in range(4):
            emit_group(12 + g2)

        # ---- softmax tail over the 7-class blocks ----
        lgs_t = tail_pool.tile([4 * C, 2 * NG], F32, tag="lgs")
        nc.vector.tensor_copy(lgs_t[:], logp_t[:])
        lgd_t = tail_pool.tile([4 * C, NG], F32, tag="lgd")
        nc.vector.tensor_sub(
            lgd_t[:],
            lgs_t[:].rearrange("p (g two) -> p two g", two=2)[:, 0, :],
            lgs_t[:].rearrange("p (g two) -> p two g", two=2)[:, 1, :],
        )
        e_t = tail_pool.tile([4 * C, NG], F32, tag="e")
        nc.scalar.activation(
            e_t[:], lgd_t[:], mybir.ActivationFunctionType.Exp, bias=hbb_sb[:],
        )
        # tail matmuls reuse sub-regions of the (already consumed) logits bank
        sum_p = logp_t[0:4, 0:NG]
        nc.tensor.matmul(sum_p, b7_sb[:], e_t[:], start=True, stop=True)
        ssb_t = tail_pool.tile([4, NG], F32, tag="ssb")
        nc.vector.tensor_copy(ssb_t[:], sum_p)
        bc_p = logp_t[:, NG : 2 * NG]
        nc.tensor.matmul(bc_p, b7t_sb[:], ssb_t[:], start=True, stop=True)
        rs_t = tail_pool.tile([4 * C, NG], F32, tag="rs")
        nc.vector.reciprocal(rs_t[:], bc_p)
        res_t = tail_pool.tile([4 * C, NG], F32, tag="res")
        nc.vector.tensor_mul(res_t[:], e_t[:], rs_t[:])
        nc.sync.dma_start(out.rearrange("(g bi) c -> (bi c) g", bi=4), res_t[:])

    nc.compile()
    return nc


_NC_CACHE = {}


def _get_nc(k):
    if k not in _NC_CACHE:
        _NC_CACHE[k] = _build_nc(k)
    return _NC_CACHE[k]


def _f32(x):
    return np.asarray(x, dtype=np.float32)


def _prepare(real, graph_sigs, W, conv_bias, pool_w, pool_b, head_w, head_b):
    real = _f32(real)
    graph_sigs = _f32(graph_sigs)
    W = _f32(W)
    conv_bias = _f32(conv_bias)
    pool_w = _f32(pool_w)
    head_w = _f32(head_w)
    head_b = _f32(head_b)

    # permute feature columns: non-negative pool_w first; fold |pool_w| into W
    nonneg = pool_w >= 0
    perm = np.argsort(~nonneg, kind="stable")
    k = int(nonneg.sum())
    apw = np.abs(pool_w)[perm]
    Wp = np.ascontiguousarray((W[:, perm] * apw[None, :]).astype(F16NP))
    cbp = (conv_bias[perm] * apw).astype(np.float32)

    wt = np.ascontiguousarray(Wp.reshape(4, 128, D).transpose(1, 0, 2))

    # DoubleRow selectors: m = 4*u + 2*h + (p//64)
    ones_dr = np.zeros((2, 64, 4, 2, 16), dtype=F8NP)
    for c in range(2):
        for u in range(4):
            for h in range(2):
                ones_dr[c, :, u, h, 4 * u + 2 * h + c] = F8NP(1.0 / T)
    ones_dr = ones_dr.reshape(128, 128)

    cbfull = np.zeros((128, D), dtype=np.float32)
    for b in range(4):
        cbfull[NP * b + N, :] = cbp

    hwblk = np.zeros((128, 4 * C), dtype=np.float32)
    for b in range(4):
        hwblk[NP * b : NP * b + N, C * b : C * (b + 1)] = head_w.T
    hb_eff = head_b + np.float32(np.asarray(pool_b)) * head_w.sum(axis=1)
    hbb = np.tile(hb_eff, 4).reshape(4 * C, 1).astype(np.float32)

    b7 = np.zeros((4 * C, 4), dtype=np.float32)
    for b in range(4):
        b7[C * b : C * (b + 1), b] = 1.0
    b7t = np.ascontiguousarray(b7.T)

    wo = np.concatenate(
        [wt.reshape(128, 4 * D), ones_dr.view(np.uint8).view(F16NP)], axis=1
    )
    cbhw = np.concatenate([cbfull, hwblk], axis=1)
    h5 = np.concatenate([hbb, b7], axis=1)
    consts = {"wo": wo, "cbhw": cbhw, "h5": h5, "b7t": b7t}
    gs_bf = graph_sigs.astype(F8NP)
    in_maps = []
    for c in range(NCORES):
        s = slice(c * BL, (c + 1) * BL)
        rt = real[s].transpose(2, 0, 1)                      # [512, BL, 30]
        rtp = np.zeros((F_IN, BL, NP), dtype=F16NP)
        rtp[:, :, :N] = rt
        gsc = np.ascontiguousarray(
            gs_bf[s]
            .reshape(8, 2, 2, 2, T, NN)
            .transpose(0, 3, 4, 2, 1, 5)
            .reshape(8, 128, 2, 2, NN)
        )
        in_maps.append(
            {
                "gs": gsc,
                "rtp": rtp.reshape(F_IN, BL * NP),
                **consts,
            }
        )
    return in_maps, k


def kernel(real, imag, graph_sigs, W, conv_bias, pool_w, pool_b, head_w, head_b):
    del imag  # unused by the forward pass
    in_maps, k = _prepare(
        real, graph_sigs, W, conv_bias, pool_w, pool_b, head_w, head_b
    )
    nc = _get_nc(k)
    res = run_bass_kernel_spmd(nc, in_maps, core_ids=list(range(NCORES)))
    return np.concatenate([res.results[c]["out"] for c in range(NCORES)], axis=0)


# revision 32
# speedup vs baseline: 2.0460x; 1.0589x over previous
"""DiGCNNet forward on 8 Trainium2 NeuronCores, data-parallel over batch.

Math (per batch b):
  adj = mean_t graph_sigs[b]                  # [30, 30]
  xw  = real[b] @ W                           # [30, 256]
  agg = adj^T @ xw + conv_bias                # [30, 256]
  h   = relu(agg)
  ns  = h @ pool_w + pool_b                   # [30]
  lg  = ns @ head_w^T + head_b                # [7]
  out = softmax(lg)

V2 design (64 batches/core, 16 gtiles of 4 batches, 4 quarters):
  - gs shipped bf16 from host (halves dominant DMA traffic; rel err ~4.5e-3).
  - T-reduce: PE matmul with per-gtile selector weights ones8 [128, 8]
    accumulating 4 gtiles into ONE PSUM tile [8, 1800] (4 banks), so the
    PSUM->SBUF copy is a single [8, 1800] op per quarter (not 16x [2,900]).
  - adjacency scatter: ONE reshape DMA per group (HWDGE, not gpsimd SWDGE):
    adjs[2 rows, 1800] -> Ablk[128part = (bo,bi,i), 32cols] fp32.
  - agg: per-batch matmuls on 32-aligned PE quadrants, K=31 row appends the
    conv_bias via a constant ones row in Ablk + bias rows in xwb (bias rows
    injected by the xwb PSUM->SBUF tensor_add with cbfull).
  - pool: W pre-scaled by |pool_w| and column-permuted (positives first) on
    host; relu+accum_out on ACT gives nsP/nsN per partition; ns = nsP-nsN
    realized inside the head matmul (rhs [128, 2], out [28, 2] per group).
  - head bias: folded into the exp() activation bias (per-partition AP).
  - softmax tail on [28, 16] once at the end.
"""

from contextlib import ExitStack

import numpy as np

import concourse.bacc as bacc
import concourse.bass as bass
import concourse.tile as tile
from concourse import mybir
from concourse.bass_utils import run_bass_kernel_spmd

F32 = mybir.dt.float32
F32R = mybir.dt.float32r
F16 = mybir.dt.float16
F8 = mybir.dt.float8e4
F16NP = np.float16
import ml_dtypes
F8NP = ml_dtypes.float8_e4m3

B, T, N = 512, 64, 30
F_IN, D, C = 512, 256, 7
NCORES = 8
BL = B // NCORES        # 64 batches per core
NN = N * N              # 900
NT = 16                 # 4-batch gtiles per core
NG = 16                 # groups of 4 batches
NP = 32                 # padded per-batch stride (partitions / realt cols)


def _build_nc(k):
    """k = number of (permuted-first) non-negative pool_w columns."""
    assert 1 <= k <= D - 1
    nc = bacc.Bacc(None, target_bir_lowering=False)

    # gs pre-tiled fp8: [8 loads, 128=(b&1,t) part, (h, u-pair, i*30+j)]
    gs = nc.dram_tensor("gs", (8, 128, 2, 2, NN), F8, kind="ExternalInput")
    rtp = nc.dram_tensor("rtp", (F_IN, BL * NP), F16, kind="ExternalInput")
    wo = nc.dram_tensor("wo", (128, 4 * D + 64), F16, kind="ExternalInput")
    cbhw = nc.dram_tensor("cbhw", (128, D + 4 * C), F32, kind="ExternalInput")
    h5 = nc.dram_tensor("h5", (4 * C, 5), F32, kind="ExternalInput")
    b7t = nc.dram_tensor("b7t", (4, 4 * C), F32, kind="ExternalInput")
    out = nc.dram_tensor("out", (BL, C), F32, kind="ExternalOutput")

    with tile.TileContext(nc) as tc, ExitStack() as ctx:
        consts = ctx.enter_context(tc.tile_pool(name="consts", bufs=1))
        gt_pool = ctx.enter_context(tc.tile_pool(name="gt", bufs=12))

        xwb_pool = ctx.enter_context(tc.tile_pool(name="xwb", bufs=2))
        h_pool = ctx.enter_context(tc.tile_pool(name="h", bufs=2))
        ns_pool = ctx.enter_context(tc.tile_pool(name="ns", bufs=2))
        tail_pool = ctx.enter_context(tc.tile_pool(name="tail", bufs=1))
        adjp_pool = ctx.enter_context(
            tc.tile_pool(name="adjp", bufs=1, space=bass.MemorySpace.PSUM)
        )
        xwp_pool = ctx.enter_context(
            tc.tile_pool(name="xwp", bufs=2, space=bass.MemorySpace.PSUM)
        )
        aggp_pool = ctx.enter_context(
            tc.tile_pool(name="aggp", bufs=2, space=bass.MemorySpace.PSUM)
        )
        logp_pool = ctx.enter_context(
            tc.tile_pool(name="logp", bufs=1, space=bass.MemorySpace.PSUM)
        )

        def load_const(dram, shape, dtype):
            t = consts.tile(shape, dtype, tag=dram.name)
            src = dram[:].bitcast(dtype) if dtype is F32R else dram[:]
            nc.scalar.dma_start(t[:], src)
            return t

        # small consts first (the first T-reduce matmul needs ones16);
        # the big rt load follows on the same queue
        wo_sb = load_const(wo, [128, 4 * D + 64], F16)
        cbhw_sb = load_const(cbhw, [128, D + 4 * C], F32)
        h5_sb = load_const(h5, [4 * C, 5], F32)
        b7t_sb = load_const(b7t, [4, 4 * C], F32)
        wt_sb = wo_sb[:, 0 : 4 * D].rearrange("p (c d) -> p c d", c=4)
        ones_dr = wo_sb[:, 4 * D : 4 * D + 64].bitcast(F8).rearrange(
            "p (u h m) -> p u h m", u=4, h=2
        )
        cb_sb = cbhw_sb[:, 0:D]
        hw_sb = cbhw_sb[:, D : D + 4 * C]
        hbb_sb = h5_sb[:, 0:1]
        b7_sb = h5_sb[:, 1:5]

        # progressive quarter-chunks: group g needs only cols 128g:128g+128,
        # so early groups can start before the whole tensor lands
        rt_sb = consts.tile([128, 4, BL * NP], F16, tag="rt")
        rtp_r = rtp.rearrange("(c p) m -> p c m", p=128)
        for rc in range(4):
            m0, m1 = 512 * rc, 512 * (rc + 1)
            nc.scalar.dma_start(rt_sb[:, :, m0:m1], rtp_r[:, :, m0:m1])

        # block-adjacency store: [128, 16 groups, 32]; all-ones memset gives
        # finite pad cols (30,31) for the M=32 agg matmuls
        ablk = consts.tile([128, NG, NP], F16, tag="ablk")
        nc.vector.memset(ablk[:], 1.0)

        # adjacency staging rows, padded to 32 elem-slots per i so the
        # per-group reshape DMA is a clean 2D->2D partition split; the pad
        # slots (1.0) become the K=31 conv_bias ones-rows in ablk
        adjs_a = consts.tile([16, NP * N], F16, tag="adjs_a")
        adjs_b = consts.tile([16, NP * N], F16, tag="adjs_b")
        nc.vector.memset(adjs_a[:, NN : NP * N], 1.0)
        nc.vector.memset(adjs_b[:, NN : NP * N], 1.0)

        logp_t = logp_pool.tile([4 * C, 2 * NG], F32, tag="logits")

        gtiles = []

        def emit_loads(q):
            for v in range(2):
                gt = gt_pool.tile([128, 2, 2, NN], F8, tag="gt")
                nc.sync.dma_start(gt[:], gs[2 * q + v])
                gtiles.append(gt)

        def emit_tred(q, u2, adjp_t):
            gt = gtiles[2 * q + u2 // 2]
            uu = u2 % 2
            for c0, c1 in ((0, 512), (512, NN)):
                nc.tensor.matmul(
                    adjp_t[:, c0:c1], ones_dr[:, u2, :, :], gt[:, :, uu, c0:c1],
                    start=(u2 == 0), stop=(u2 == 3),
                    perf_mode=mybir.MatmulPerfMode.DoubleRow,
                )

        def emit_adj_finish(q, adjp_t):
            adjs_t = adjs_a if q % 2 == 0 else adjs_b
            if q % 2 == 0:
                nc.scalar.copy(adjs_t[:, 0:NN], adjp_t[:])
            else:
                nc.vector.tensor_copy(adjs_t[:, 0:NN], adjp_t[:])
            for g2 in range(4):
                g = 4 * q + g2
                nc.gpsimd.dma_start(ablk[:, g, 0:N], adjs_t[4 * g2 : 4 * g2 + 4, :])

        def emit_group(g):
            xwp_t = xwp_pool.tile([128, D], F32, tag="xwp")
            for c4 in range(4):
                nc.tensor.matmul(
                    xwp_t[:], rt_sb[:, c4, 128 * g : 128 * (g + 1)], wt_sb[:, c4, :],
                    start=(c4 == 0), stop=(c4 == 3),
                )
            xwb_t = xwb_pool.tile([128, D], F16, tag="xwb")
            nc.vector.tensor_add(xwb_t[:], xwp_t[:], cb_sb[:])

            aggp_t = aggp_pool.tile([128, D], F32, tag="aggp")
            for b in range(4):
                p0 = NP * b
                nc.tensor.matmul(
                    aggp_t[p0 : p0 + NP, :],
                    ablk[p0 : p0 + 31, g, :],
                    xwb_t[p0 : p0 + 31, :],
                    start=True, stop=True, tile_position=(p0, p0),
                )
            h_t = h_pool.tile([128, D], F32, tag="h")
            ns_t = ns_pool.tile([128, 2], F32, tag="ns")
            nc.scalar.activation(
                h_t[:, 0:k], aggp_t[:, 0:k],
                mybir.ActivationFunctionType.Relu, accum_out=ns_t[:, 0:1],
            )
            nc.vector.tensor_scalar(
                h_t[:, k:D], aggp_t[:, k:D], 0.0, 0.0,
                mybir.AluOpType.max, mybir.AluOpType.add,
                accum_out=ns_t[:, 1:2],
            )
            nc.tensor.matmul(
                logp_t[:, 2 * g : 2 * g + 2], hw_sb[:], ns_t[:],
                start=True, stop=True,
            )

        # ---- pipelined emission ----
        emit_loads(0)
        emit_loads(1)
        for q in range(4):
            adjp_t = adjp_pool.tile([16, NN], F32, tag="adjp")
            for u2 in range(4):
                emit_tred(q, u2, adjp_t)
                if q >= 1:
                    emit_group(4 * (q - 1) + u2)
            emit_adj_finish(q, adjp_t)
            if q < 2:
                emit_loads(q + 2)
        for g2 in range(4):
            emit_group(12 + g2)

        # ---- softmax tail over the 7-class blocks ----
        lgs_t = tail_pool.tile([4 * C, 2 * NG], F32, tag="lgs")
        nc.vector.tensor_copy(lgs_t[:], logp_t[:])
        lgd_t = tail_pool.tile([4 * C, NG], F32, tag="lgd")
        nc.vector.tensor_sub(
            lgd_t[:],
            lgs_t[:].rearrange("p (g two) -> p two g", two=2)[:, 0, :],
            lgs_t[:].rearrange("p (g two) -> p two g", two=2)[:, 1, :],
        )
        e_t = tail_pool.tile([4 * C, NG], F32, tag="e")
        nc.scalar.activation(
            e_t[:], lgd_t[:], mybir.ActivationFunctionType.Exp, bias=hbb_sb[:],
        )
        # tail matmuls reuse sub-regions of the (already consumed) logits bank
        sum_p = logp_t[0:4, 0:NG]
        nc.tensor.matmul(sum_p, b7_sb[:], e_t[:], start=True, stop=True)
        ssb_t = tail_pool.tile([4, NG], F32, tag="ssb")
        nc.vector.tensor_copy(ssb_t[:], sum_p)
        bc_p = logp_t[:, NG : 2 * NG]
        nc.tensor.matmul(bc_p, b7t_sb[:], ssb_t[:], start=True, stop=True)
        rs_t = tail_pool.tile([4 * C, NG], F32, tag="rs")
        nc.vector.reciprocal(rs_t[:], bc_p)
        res_t = tail_pool.tile([4 * C, NG], F32, tag="res")
        nc.vector.tensor_mul(res_t[:], e_t[:], rs_t[:])
        nc.sync.dma_start(out.rearrange("(g bi) c -> (bi c) g", bi=4), res_t[:])

    nc.compile()
    return nc


_NC_CACHE = {}


def _get_nc(k):
    if k not in _NC_CACHE:
        _NC_CACHE[k] = _build_nc(k)
    return _NC_CACHE[k]


def _f32(x):
    return np.asarray(x, dtype=np.float32)


def _prepare(real, graph_sigs, W, conv_bias, pool_w, pool_b, head_w, head_b):
    real = _f32(real)
    graph_sigs = _f32(graph_sigs)
    W = _f32(W)
    conv_bias = _f32(conv_bias)
    pool_w = _f32(pool_w)
    head_w = _f32(head_w)
    head_b = _f32(head_b)

    # permute feature columns: non-negative pool_w first; fold |pool_w| into W
    nonneg = pool_w >= 0
    perm = np.argsort(~nonneg, kind="stable")
    k = int(nonneg.sum())
    apw = np.abs(pool_w)[perm]
    Wp = np.ascontiguousarray((W[:, perm] * apw[None, :]).astype(F16NP))
    cbp = (conv_bias[perm] * apw).astype(np.float32)

    wt = np.ascontiguousarray(Wp.reshape(4, 128, D).transpose(1, 0, 2))

    # DoubleRow selectors: m = 4*u + 2*h + (p//64)
    ones_dr = np.zeros((2, 64, 4, 2, 16), dtype=F8NP)
    for c in range(2):
        for u in range(4):
            for h in range(2):
                ones_dr[c, :, u, h, 4 * u + 2 * h + c] = F8NP(1.0 / T)
    ones_dr = ones_dr.reshape(128, 128)

    cbfull = np.zeros((128, D), dtype=np.float32)
    for b in range(4):
        cbfull[NP * b + N, :] = cbp

    hwblk = np.zeros((128, 4 * C), dtype=np.float32)
    for b in range(4):
        hwblk[NP * b : NP * b + N, C * b : C * (b + 1)] = head_w.T
    hb_eff = head_b + np.float32(np.asarray(pool_b)) * head_w.sum(axis=1)
    hbb = np.tile(hb_eff, 4).reshape(4 * C, 1).astype(np.float32)

    b7 = np.zeros((4 * C, 4), dtype=np.float32)
    for b in range(4):
        b7[C * b : C * (b + 1), b] = 1.0
    b7t = np.ascontiguousarray(b7.T)

    wo = np.concatenate(
        [wt.reshape(128, 4 * D), ones_dr.view(np.uint8).view(F16NP)], axis=1
    )
    cbhw = np.concatenate([cbfull, hwblk], axis=1)
    h5 = np.concatenate([hbb, b7], axis=1)
    consts = {"wo": wo, "cbhw": cbhw, "h5": h5, "b7t": b7t}
    gs_bf = graph_sigs.astype(F8NP)
    in_maps = []
    for c in range(NCORES):
        s = slice(c * BL, (c + 1) * BL)
        rt = real[s].transpose(2, 0, 1)                      # [512, BL, 30]
        rtp = np.zeros((F_IN, BL, NP), dtype=F16NP)
        rtp[:, :, :N] = rt
        gsc = np.ascontiguousarray(
            gs_bf[s]
            .reshape(8, 2, 2, 2, T, NN)
            .transpose(0, 3, 4, 2, 1, 5)
            .reshape(8, 128, 2, 2, NN)
        )
        in_maps.append(
            {
                "gs": gsc,
                "rtp": rtp.reshape(F_IN, BL * NP),
                **consts,
            }
        )
    return in_maps, k


def kernel(real, imag, graph_sigs, W, conv_bias, pool_w, pool_b, head_w, head_b):
    del imag  # unused by the forward pass
    in_maps, k = _prepare(
        real, graph_sigs, W, conv_bias, pool_w, pool_b, head_w, head_b
    )
    nc = _get_nc(k)
    res = run_bass_kernel_spmd(nc, in_maps, core_ids=list(range(NCORES)))
    return np.concatenate([res.results[c]["out"] for c in range(NCORES)], axis=0)


# revision 36
# speedup vs baseline: 2.0713x; 1.0124x over previous
"""DiGCNNet forward on 8 Trainium2 NeuronCores, data-parallel over batch.

Math (per batch b):
  adj = mean_t graph_sigs[b]                  # [30, 30]
  xw  = real[b] @ W                           # [30, 256]
  agg = adj^T @ xw + conv_bias                # [30, 256]
  h   = relu(agg)
  ns  = h @ pool_w + pool_b                   # [30]
  lg  = ns @ head_w^T + head_b                # [7]
  out = softmax(lg)

V6 design (64 batches/core, 16 groups of 4, 8 half-quarters of 8):
  - gs shipped fp8e4m3 (quarter DMA traffic); T-reduce as fp8 DoubleRow
    matmuls (K=256: 4 batches per mm) accumulating 2 loads into a PSUM tile
    [8, 900] per half-quarter; rel err ~8e-3 (tolerance 2e-2).
  - real shipped fp16, padded to 32 cols/batch, as 4 independent tiles so
    early groups don't wait on the whole tensor.
  - adjacency: one [8,900] PSUM->SBUF copy per half-quarter into padded
    [8, 960] staging rows (pad slots pre-set to 1.0), then ONE gpsimd
    reshape DMA per group -> ablk[128, g, 32] (the 1.0 pads become the
    K=31 conv_bias ones-rows).
  - agg: per-batch fp16 matmuls on 32-aligned PE quadrants; conv_bias rows
    injected into xwb by the PSUM->SBUF tensor_add with cbfull.
  - pool: W pre-scaled by |pool_w|, columns permuted positives-first; ACT
    relu+accum gives nsP, DVE max+accum gives nsN; head matmuls (batched at
    the end) compute nsP@hw - nsN@hw via rhs [128, 2] -> out [28, 2].
  - head bias folded into the exp() bias AP; softmax tail on [28, 16] once.
"""

from contextlib import ExitStack

import numpy as np
import ml_dtypes

import concourse.bacc as bacc
import concourse.bass as bass
import concourse.tile as tile
from concourse import mybir
from concourse.bass_utils import run_bass_kernel_spmd

F32 = mybir.dt.float32
F16 = mybir.dt.float16
F8 = mybir.dt.float8e4
F16NP = np.float16
F8NP = ml_dtypes.float8_e4m3

B, T, N = 512, 64, 30
F_IN, D, C = 512, 256, 7
NCORES = 8
BL = B // NCORES        # 64 batches per core
NN = N * N              # 900
NG = 16                 # groups of 4 batches
NP = 32                 # padded per-batch stride (partitions / realt cols)


def _build_nc(k):
    """k = number of (permuted-first) non-negative pool_w columns."""
    assert 1 <= k <= D - 1
    nc = bacc.Bacc(None, target_bir_lowering=False)

    # gs pre-tiled fp8: [8 loads, 128=(b&1,t) part, (h, u-pair, i*30+j)]
    gs = nc.dram_tensor("gs", (8, 128, 2, 2, NN), F8, kind="ExternalInput")
    rtp = nc.dram_tensor("rtp", (4, F_IN, 512), F16, kind="ExternalInput")
    wo = nc.dram_tensor("wo", (128, 4 * D + 64), F16, kind="ExternalInput")
    cbhw = nc.dram_tensor("cbhw", (128, D + 4 * C), F32, kind="ExternalInput")
    h5 = nc.dram_tensor("h5", (4 * C, 5), F32, kind="ExternalInput")
    b7t = nc.dram_tensor("b7t", (4, 4 * C), F32, kind="ExternalInput")
    out = nc.dram_tensor("out", (BL, C), F32, kind="ExternalOutput")

    with tile.TileContext(nc) as tc, ExitStack() as ctx:
        consts = ctx.enter_context(tc.tile_pool(name="consts", bufs=1))
        gt_pool = ctx.enter_context(tc.tile_pool(name="gt", bufs=8))
        xwb_pool = ctx.enter_context(tc.tile_pool(name="xwb", bufs=2))
        h_pool = ctx.enter_context(tc.tile_pool(name="h", bufs=2))
        tail_pool = ctx.enter_context(tc.tile_pool(name="tail", bufs=1))
        adjp_pool = ctx.enter_context(
            tc.tile_pool(name="adjp", bufs=1, space=bass.MemorySpace.PSUM)
        )
        xwp_pool = ctx.enter_context(
            tc.tile_pool(name="xwp", bufs=2, space=bass.MemorySpace.PSUM)
        )
        aggp_pool = ctx.enter_context(
            tc.tile_pool(name="aggp", bufs=2, space=bass.MemorySpace.PSUM)
        )
        logp_pool = ctx.enter_context(
            tc.tile_pool(name="logp", bufs=1, space=bass.MemorySpace.PSUM)
        )

        def load_const(dram, shape, dtype):
            t = consts.tile(shape, dtype, tag=dram.name)
            nc.scalar.dma_start(t[:], dram[:])
            return t

        # small consts first (the first T-reduce needs the selectors packed
        # into wo); the rt quarter-tiles follow on the same queue
        wo_sb = load_const(wo, [128, 4 * D + 64], F16)
        cbhw_sb = load_const(cbhw, [128, D + 4 * C], F32)
        h5_sb = load_const(h5, [4 * C, 5], F32)
        b7t_sb = load_const(b7t, [4, 4 * C], F32)
        wt_sb = wo_sb[:, 0 : 4 * D].rearrange("p (c d) -> p c d", c=4)
        ones_dr = wo_sb[:, 4 * D : 4 * D + 64].bitcast(F8).rearrange(
            "p (u h m) -> p u h m", u=4, h=2
        )
        cb_sb = cbhw_sb[:, 0:D]
        hw_sb = cbhw_sb[:, D : D + 4 * C]
        hbb_sb = h5_sb[:, 0:1]
        b7_sb = h5_sb[:, 1:5]

        rtq = []
        for rc in range(4):
            t = consts.tile([128, 4, 512], F16, tag=f"rtq{rc}")
            nc.scalar.dma_start(t[:], rtp[rc].rearrange("(c p) m -> p c m", p=128))
            rtq.append(t)

        # block-adjacency store: [128, 16 groups, 32]; all-ones memset gives
        # finite pad cols (30,31) for the M=32 agg matmuls
        ablk = consts.tile([128, NG, NP], F16, tag="ablk")
        nc.vector.memset(ablk[:], 1.0)

        # adjacency staging rows, padded to 32 elem-slots per i so the
        # per-group reshape DMA is a clean 2D->2D partition split; the pad
        # slots (1.0) become the K=31 conv_bias ones-rows in ablk
        adjs_a = consts.tile([16, NP * N], F16, tag="adjs_a")
        adjs_b = consts.tile([16, NP * N], F16, tag="adjs_b")
        nc.vector.memset(adjs_a[:, NN : NP * N], 1.0)
        nc.vector.memset(adjs_b[:, NN : NP * N], 1.0)

        ns_all = consts.tile([128, NG, 2], F32, tag="ns_all")
        logp_t = logp_pool.tile([4 * C, 2 * NG], F32, tag="logits")

        gtiles = []
        for v in range(8):
            gt = gt_pool.tile([128, 2, 2, NN], F8, tag="gt")
            nc.sync.dma_start(gt[:], gs[v])
            gtiles.append(gt)

        def emit_tred(q, u2, adjp_t):
            gt = gtiles[2 * q + u2 // 2]
            uu = u2 % 2
            for c0, c1 in ((0, 512), (512, NN)):
                nc.tensor.matmul(
                    adjp_t[:, c0:c1], ones_dr[:, u2, :, :], gt[:, :, uu, c0:c1],
                    start=(u2 == 0), stop=(u2 == 3),
                    perf_mode=mybir.MatmulPerfMode.DoubleRow,
                )

        def emit_adj_finish(q, adjp_t):
            adjs_t = adjs_a if q % 2 == 0 else adjs_b
            if q % 2 == 0:
                nc.scalar.copy(adjs_t[:, 0:NN], adjp_t[:])
            else:
                nc.vector.tensor_copy(adjs_t[:, 0:NN], adjp_t[:])
            for g2 in range(4):
                g = 4 * q + g2
                nc.gpsimd.dma_start(
                    ablk[:, g, 0:N], adjs_t[4 * g2 : 4 * g2 + 4, :]
                )

        def emit_group(g):
            xwp_t = xwp_pool.tile([128, D], F32, tag="xwp")
            for c4 in range(4):
                nc.tensor.matmul(
                    xwp_t[:],
                    rtq[g // 4][:, c4, 128 * (g % 4) : 128 * (g % 4 + 1)],
                    wt_sb[:, c4, :],
                    start=(c4 == 0), stop=(c4 == 3),
                )
            xwb_t = xwb_pool.tile([128, D], F16, tag="xwb")
            nc.vector.tensor_add(xwb_t[:], xwp_t[:], cb_sb[:])

            aggp_t = aggp_pool.tile([128, D], F32, tag="aggp")
            for b in range(4):
                p0 = NP * b
                nc.tensor.matmul(
                    aggp_t[p0 : p0 + NP, :],
                    ablk[p0 : p0 + 31, g, :],
                    xwb_t[p0 : p0 + 31, :],
                    start=True, stop=True, tile_position=(p0, p0),
                )
            h_t = h_pool.tile([128, D], F32, tag="h")
            nc.scalar.activation(
                h_t[:, 0:k], aggp_t[:, 0:k],
                mybir.ActivationFunctionType.Relu, accum_out=ns_all[:, g, 0:1],
            )
            nc.vector.tensor_scalar(
                h_t[:, k:D], aggp_t[:, k:D], 0.0, 0.0,
                mybir.AluOpType.max, mybir.AluOpType.add,
                accum_out=ns_all[:, g, 1:2],
            )

        # ---- pipelined emission: 4 quarters of 16 batches ----
        for q in range(4):
            adjp_t = adjp_pool.tile([16, NN], F32, tag="adjp")
            for u2 in range(4):
                emit_tred(q, u2, adjp_t)
                if q >= 1:
                    emit_group(4 * (q - 1) + u2)
            emit_adj_finish(q, adjp_t)
        for g2 in range(4):
            emit_group(12 + g2)

        # ---- head matmuls batched: lg[28, 2] per group ----
        for g in range(NG):
            nc.tensor.matmul(
                logp_t[:, 2 * g : 2 * g + 2], hw_sb, ns_all[:, g, :],
                start=True, stop=True,
            )

        # ---- softmax tail over the 7-class blocks ----
        lgs_t = tail_pool.tile([4 * C, 2 * NG], F32, tag="lgs")
        nc.vector.tensor_copy(lgs_t[:], logp_t[:])
        lgd_t = tail_pool.tile([4 * C, NG], F32, tag="lgd")
        nc.vector.tensor_sub(
            lgd_t[:],
            lgs_t[:].rearrange("p (g two) -> p two g", two=2)[:, 0, :],
            lgs_t[:].rearrange("p (g two) -> p two g", two=2)[:, 1, :],
        )
        e_t = tail_pool.tile([4 * C, NG], F32, tag="e")
        nc.scalar.activation(
            e_t[:], lgd_t[:], mybir.ActivationFunctionType.Exp, bias=hbb_sb,
        )
        # tail matmuls reuse sub-regions of the (already consumed) logits bank
        sum_p = logp_t[0:4, 0:NG]
        nc.tensor.matmul(sum_p, b7_sb, e_t[:], start=True, stop=True)
        ssb_t = tail_pool.tile([4, NG], F32, tag="ssb")
        nc.vector.tensor_copy(ssb_t[:], sum_p)
        bc_p = logp_t[:, NG : 2 * NG]
        nc.tensor.matmul(bc_p, b7t_sb[:], ssb_t[:], start=True, stop=True)
        rs_t = tail_pool.tile([4 * C, NG], F32, tag="rs")
        nc.vector.reciprocal(rs_t[:], bc_p)
        res_t = tail_pool.tile([4 * C, NG], F32, tag="res")
        nc.vector.tensor_mul(res_t[:], e_t[:], rs_t[:])
        nc.sync.dma_start(out.rearrange("(g bi) c -> (bi c) g", bi=4), res_t[:])

    nc.compile()
    return nc


_NC_CACHE = {}


def _get_nc(k):
    if k not in _NC_CACHE:
        _NC_CACHE[k] = _build_nc(k)
    return _NC_CACHE[k]


def _f32(x):
    return np.asarray(x, dtype=np.float32)


def _prepare(real, graph_sigs, W, conv_bias, pool_w, pool_b, head_w, head_b):
    real = _f32(real)
    graph_sigs = _f32(graph_sigs)
    W = _f32(W)
    conv_bias = _f32(conv_bias)
    pool_w = _f32(pool_w)
    head_w = _f32(head_w)
    head_b = _f32(head_b)

    # permute feature columns: non-negative pool_w first; fold |pool_w| into W
    nonneg = pool_w >= 0
    perm = np.argsort(~nonneg, kind="stable")
    k = int(nonneg.sum())
    apw = np.abs(pool_w)[perm]
    Wp = np.ascontiguousarray((W[:, perm] * apw[None, :]).astype(F16NP))
    cbp = (conv_bias[perm] * apw).astype(np.float32)

    wt = np.ascontiguousarray(Wp.reshape(4, 128, D).transpose(1, 0, 2))

    # DoubleRow selectors: m = 4*u + 2*h + (p//64), u = within-load half
    ones_dr = np.zeros((2, 64, 4, 2, 16), dtype=F8NP)
    for c in range(2):
        for u in range(4):
            for h in range(2):
                ones_dr[c, :, u, h, 4 * u + 2 * h + c] = F8NP(1.0 / T)
    ones_dr = ones_dr.reshape(128, 128)

    cbfull = np.zeros((128, D), dtype=np.float32)
    for b in range(4):
        cbfull[NP * b + N, :] = cbp

    hwblk = np.zeros((128, 4 * C), dtype=np.float32)
    for b in range(4):
        hwblk[NP * b : NP * b + N, C * b : C * (b + 1)] = head_w.T
    hb_eff = head_b + np.float32(np.asarray(pool_b)) * head_w.sum(axis=1)
    hbb = np.tile(hb_eff, 4).reshape(4 * C, 1).astype(np.float32)

    b7 = np.zeros((4 * C, 4), dtype=np.float32)
    for b in range(4):
        b7[C * b : C * (b + 1), b] = 1.0
    b7t = np.ascontiguousarray(b7.T)

    wo = np.concatenate(
        [wt.reshape(128, 4 * D), ones_dr.view(np.uint8).view(F16NP)], axis=1
    )
    cbhw = np.concatenate([cbfull, hwblk], axis=1)
    h5 = np.concatenate([hbb, b7], axis=1)
    consts = {"wo": wo, "cbhw": cbhw, "h5": h5, "b7t": b7t}

    gs_8 = graph_sigs.astype(F8NP)
    in_maps = []
    for c in range(NCORES):
        s = slice(c * BL, (c + 1) * BL)
        rt = real[s].transpose(2, 0, 1)                      # [512, BL, 30]
        rtp = np.zeros((F_IN, BL, NP), dtype=F16NP)
        rtp[:, :, :N] = rt
        gsc = np.ascontiguousarray(
            gs_8[s]
            .reshape(8, 2, 2, 2, T, NN)
            .transpose(0, 3, 4, 2, 1, 5)
            .reshape(8, 128, 2, 2, NN)
        )
        in_maps.append(
            {
                "gs": gsc,
                "rtp": np.ascontiguousarray(
                    rtp.reshape(F_IN, 4, 512).transpose(1, 0, 2)
                ),
                **consts,
            }
        )
    return in_maps, k


def kernel(real, imag, graph_sigs, W, conv_bias, pool_w, pool_b, head_w, head_b):
    del imag  # unused by the forward pass
    in_maps, k = _prepare(
        real, graph_sigs, W, conv_bias, pool_w, pool_b, head_w, head_b
    )
    nc = _get_nc(k)
    res = run_bass_kernel_spmd(nc, in_maps, core_ids=list(range(NCORES)))
    return np.concatenate([res.results[c]["out"] for c in range(NCORES)], axis=0)
